# revision 1
# baseline (speedup 1.0000x reference)
"""2-layer GCN on 8 Trainium2 NeuronCores (Bass/Tile).

out = log_softmax( A @ (relu(A @ (x@W1) + b1) @ W2) + b2 ),  A sparse COO (rows sorted).

Strategy (1D row partition, per sharding hint):
- Host relabels nodes (permutation pi) so that every 32-row "window" has a
  balanced number of incoming edges from each table "quarter" (the int16
  dma_gather index limit forces gathering from <=32767-row subtables, so the
  feature table is split into 4 quarters = contiguous slot ranges).
- Each core owns `STRIPS` strips of 128 node-slots. Edges are packed into
  fixed 128-slot chunks: chunk (strip, src-quarter q, window w, j). Gathers
  are one dma_gather per (strip, q) on SWDGE queue q; segment-sum is done on
  the tensor engine: psum[32w:32w+32, :] += S_chunk.T @ gathered_chunk where
  S[e, r] = val[e] * (rowrel[e] == r) is built on the vector engine from
  per-edge (rowrel, val) via broadcast-AP compare against an iota constant.
- Dense stages: support = x@W1 (f32 matmuls, host-transposed x shard);
  t2 = h @ W2pad via per-strip PE transpose; log_softmax on ACT/DVE.
- Full feature tables are replicated per-core via AllGather (bf16, 256B rows).
"""

import numpy as np
import ml_dtypes

BF16 = ml_dtypes.bfloat16
P = 128
WIN = 32

# problem constants (hardcoded per spec)
N_NODES = 100000
N_EDGES = 3200000
NFEAT = 256
NHID = 128
NCLASS = 40

DEFAULT_CFG = dict(n_cores=8, strips=108, J=2)


# ----------------------------------------------------------------------------
# host preprocessing
# ----------------------------------------------------------------------------

def _balance_pi(adj_row, adj_col, n_nodes, cfg):
    """Assign nodes to slots. Returns (pi[n_nodes] -> slot, J)."""
    n_cores, strips = cfg["n_cores"], cfg["strips"]
    total_strips = n_cores * strips
    assert total_strips % 4 == 0
    nslot = total_strips * P
    qsize = nslot // 4
    wpq = qsize // WIN               # windows per quarter
    assert qsize <= 32767

    deg = np.bincount(adj_row, minlength=n_nodes).astype(np.int64)

    # phase (a): nodes -> quarters, snake by degree. quarter q = slots
    # [q*qsize, (q+1)*qsize); capacity qsize >= ceil(n_nodes/4).
    order = np.argsort(-deg, kind="stable")
    qcap = qsize
    quarter = np.full(n_nodes, -1, np.int32)
    snake = np.tile(np.array([0, 1, 2, 3, 3, 2, 1, 0], np.int32),
                    (n_nodes + 7) // 8)[:n_nodes]
    # snake respecting capacity (capacity only binds if n_nodes/4 > qsize)
    counts = np.zeros(4, np.int64)
    qa = snake.copy()
    quarter[order] = qa
    for q in range(4):
        counts[q] = (qa == q).sum()
        assert counts[q] <= qcap, (q, counts[q], qcap)

    # per-node 4-vector: incoming-edge counts by source quarter
    srcq = quarter[adj_col]
    vec = np.zeros((n_nodes, 4), np.int64)
    np.add.at(vec, (adj_row, srcq), 1)

    # phase (b): within each quarter, block-paired LPT into wpq windows
    # (32 blocks; block b pairs heaviest remaining nodes with lightest windows)
    pi = np.full(n_nodes, -1, np.int64)
    maxload = 0
    for q in range(4):
        nodes_q = np.where(quarter == q)[0]
        nq = len(nodes_q)
        ordq = nodes_q[np.argsort(-deg[nodes_q], kind="stable")]
        loads = np.zeros((wpq, 4), np.int64)
        for b in range(WIN):
            blk = ordq[b * wpq:(b + 1) * wpq]
            if len(blk) == 0:
                break
            bs = blk[np.argsort(-vec[blk].max(1), kind="stable")]
            ws = np.argsort(loads.max(1), kind="stable")[:len(bs)]
            # slot: quarter q, window ws, position b
            strip_in_q = ws // 4
            win_in_strip = ws % 4
            pi[bs] = (q * (total_strips // 4) + strip_in_q) * P \
                + win_in_strip * WIN + b
            loads[ws] += vec[bs]
        maxload = max(maxload, int(loads.max()))
    assert (pi[deg > 0] >= 0).all()
    # unassigned (shouldn't happen: every node is assigned above)
    assert (pi >= 0).all()
    J = max(2, -(-maxload // P))
    return pi, J, maxload


def _prep(x, adj_row, adj_col, adj_val, cfg):
    """Build per-core device input arrays. Returns (in_maps_extra, pi, J)."""
    n_cores, strips = cfg["n_cores"], cfg["strips"]
    n_nodes = x.shape[0]
    nfeat = x.shape[1]
    total_strips = n_cores * strips
    nslot = total_strips * P
    qsize = nslot // 4
    rows_pc = strips * P

    pi, J, maxload = _balance_pi(adj_row, adj_col, n_nodes, cfg)
    cfg = dict(cfg, J=J)
    nch = 16 * J                      # chunks per strip
    sps = nch * P                     # slots per strip
    cpq = 4 * J                       # chunks per (strip, quarter) call

    rowslot = pi[adj_row]
    colslot = pi[adj_col]
    srcq = (colslot // qsize).astype(np.int64)
    s_glob = rowslot // P
    k_win = (rowslot % P) // WIN

    seg = (s_glob * 4 + srcq) * 4 + k_win
    order = np.argsort(seg, kind="stable")
    seg_sorted = seg[order]
    seg_counts = np.bincount(seg_sorted, minlength=total_strips * 16)
    assert seg_counts.max() <= J * P, (seg_counts.max(), J * P)
    seg_off = np.zeros_like(seg_counts)
    np.cumsum(seg_counts[:-1], out=seg_off[1:])
    rank = np.arange(len(order)) - seg_off[seg_sorted]

    es = order  # edge ids in segment order
    e_s = s_glob[es]
    e_q = srcq[es]
    e_k = k_win[es]
    eslot = e_s * sps + (e_q * cpq + e_k * J) * P + rank

    idx16 = np.zeros(total_strips * sps, np.int16)
    rr8 = np.zeros(total_strips * sps, np.float32)
    valf = np.zeros(total_strips * sps, np.float32)
    idx16[eslot] = (colslot[es] % qsize).astype(np.int16)
    rr8[eslot] = (rowslot[es] % WIN).astype(np.float32)
    # pads: rr stays 0 but val 0 -> S=0, harmless
    valf[eslot] = adj_val[es]

    # per-core aux tensor [128, strips * (4*cpq*8 + 2*nch)] int16
    idx_cols = cpq * 8                # per call
    aux_cols = 4 * idx_cols + 2 * nch
    aux = np.zeros((n_cores, P, strips * aux_cols), np.int16)
    idxv = idx16.reshape(total_strips, 16 * J, P)
    rrv = rr8.reshape(total_strips, 16 * J, P)
    valv = valf.reshape(total_strips, 16 * J, P)
    for c in range(n_cores):
        for sl in range(strips):
            sg = c * strips + sl
            base = sl * aux_cols
            for q in range(4):
                call = idxv[sg, q * cpq:(q + 1) * cpq].reshape(-1)  # [cpq*128]
                blk = np.zeros((16, idx_cols), np.int16)
                n = call.shape[0]
                blk[np.arange(n) % 16, np.arange(n) // 16] = call
                col0 = base + q * idx_cols
                for rep in range(8):
                    aux[c, rep * 16:(rep + 1) * 16, col0:col0 + idx_cols] = blk
            rrt = rrv[sg].T.astype(BF16).view(np.int16)      # [128, nch]
            valt = valv[sg].T.astype(BF16).view(np.int16)
            aux[c, :, base + 4 * idx_cols: base + 4 * idx_cols + nch] = rrt
            aux[c, :, base + 4 * idx_cols + nch: base + aux_cols] = valt

    # xT shards [nfeat, rows_pc] f32, pi-permuted, virtual slots zero
    xT = np.zeros((n_cores, nfeat, rows_pc), np.float32)
    xs = np.asarray(x, np.float32)
    core_of = pi // rows_pc
    loc = pi % rows_pc
    for c in range(n_cores):
        m = core_of == c
        xT[c][:, loc[m]] = xs[m].T

    return aux, xT, pi, cfg


# ----------------------------------------------------------------------------
# device program
# ----------------------------------------------------------------------------

_PROGRAM_CACHE = {}


def _build_program(cfg):
    import concourse.bacc as bacc
    import concourse.bass as bass
    import concourse.mybir as mybir
    import concourse.tile as tile
    from concourse.masks import make_identity

    n_cores, strips, J = cfg["n_cores"], cfg["strips"], cfg["J"]
    nfeat, nhid, ncls = cfg["nfeat"], cfg["nhid"], cfg["ncls"]
    assert nhid == P
    rows_pc = strips * P
    nslot = n_cores * rows_pc
    qsize = nslot // 4
    nch = 16 * J
    cpq = 4 * J
    idx_cols = cpq * 8
    aux_cols = 4 * idx_cols + 2 * nch
    kf = nfeat // P                  # k-chunks for x@W1

    nc = bacc.Bacc("TRN2", target_bir_lowering=False, debug=False,
                   num_devices=n_cores, num_swdge_queues=4)

    aux_d = nc.dram_tensor("aux", [P, strips * aux_cols], mybir.dt.int16,
                           kind="ExternalInput")
    xT_d = nc.dram_tensor("xT", [nfeat, rows_pc], mybir.dt.float32,
                          kind="ExternalInput")
    W1_d = nc.dram_tensor("W1", [nfeat, nhid], mybir.dt.float32,
                          kind="ExternalInput")
    W2_d = nc.dram_tensor("W2", [nhid, ncls], mybir.dt.float32,
                          kind="ExternalInput")
    b1_d = nc.dram_tensor("b1", [1, nhid], mybir.dt.float32,
                          kind="ExternalInput")
    b2_d = nc.dram_tensor("b2", [1, ncls], mybir.dt.float32,
                          kind="ExternalInput")
    iota_d = nc.dram_tensor("iota", [P, WIN], mybir.dt.bfloat16,
                            kind="ExternalInput")
    out_d = nc.dram_tensor("out", [rows_pc, ncls], mybir.dt.float32,
                           kind="ExternalOutput")

    shared_kw = {"addr_space": "Shared"} if n_cores > 4 else {}
    rg = [list(range(n_cores))]

    with tile.TileContext(nc) as tc:
        with (
            tc.tile_pool(name="const", bufs=1) as cp,
            tc.tile_pool(name="sb", bufs=3) as sb,
            tc.tile_pool(name="sbS", bufs=2) as sbS,
            tc.tile_pool(name="aux", bufs=2) as sbA,
            tc.tile_pool(name="big", bufs=2) as big,
            tc.tile_pool(name="ps", bufs=2, space="PSUM") as psp,
            tc.tile_pool(name="psC", bufs=2, space="PSUM") as psC,
            tc.tile_pool(name="dram", bufs=1, space="DRAM") as dram,
        ):
            # ---------------- constants ----------------
            ident = cp.tile([P, P], mybir.dt.bfloat16, tag="id")
            make_identity(nc, ident[:])
            iota_sb = cp.tile([P, WIN], mybir.dt.bfloat16, tag="iota")
            nc.sync.dma_start(iota_sb[:], iota_d[:])
            ones_sb = cp.tile([1, P], mybir.dt.bfloat16, tag="ones")
            nc.vector.memset(ones_sb[:], 1.0)
            zrow_sb = cp.tile([1, P], mybir.dt.bfloat16, tag="zrow")
            nc.vector.memset(zrow_sb[:], 0.0)
            b1f = cp.tile([1, nhid], mybir.dt.float32, tag="b1f")
            nc.sync.dma_start(b1f[:], b1_d[:])
            b1row = cp.tile([1, nhid], mybir.dt.bfloat16, tag="b1b")
            nc.vector.tensor_copy(b1row[:], b1f[:])
            b2f = cp.tile([1, ncls], mybir.dt.float32, tag="b2f")
            nc.sync.dma_start(b2f[:], b2_d[:])
            b2row = cp.tile([1, ncls], mybir.dt.bfloat16, tag="b2b")
            nc.vector.tensor_copy(b2row[:], b2f[:])
            W2f = cp.tile([nhid, ncls], mybir.dt.float32, tag="w2f")
            nc.sync.dma_start(W2f[:], W2_d[:])
            W2pad = cp.tile([nhid, P], mybir.dt.bfloat16, tag="w2p")
            nc.vector.memset(W2pad[:], 0.0)
            nc.vector.tensor_copy(W2pad[:, 0:ncls], W2f[:])
            W1sb = []
            for k in range(kf):
                w = cp.tile([P, nhid], mybir.dt.float32, tag=f"w1_{k}")
                nc.sync.dma_start(w[:], W1_d[k * P:(k + 1) * P, :])
                W1sb.append(w)

            sup_shard = dram.tile([rows_pc, nhid], mybir.dt.bfloat16)
            table1 = dram.tile([nslot, nhid], mybir.dt.bfloat16, **shared_kw)
            t2_shard = dram.tile([rows_pc, P], mybir.dt.bfloat16)
            table2 = dram.tile([nslot, P], mybir.dt.bfloat16, **shared_kw)

            # ---------------- phase A: support = x @ W1 ----------------
            sup_all = big.tile([P, strips * nhid], mybir.dt.bfloat16, tag="big")
            for s in range(strips):
                ps = psp.tile([P, nhid], mybir.dt.float32, space="PSUM",
                              tag="psA")
                for k in range(kf):
                    xt = sb.tile([P, P], mybir.dt.float32, tag="xt")
                    nc.sync.dma_start(
                        xt[:], xT_d[k * P:(k + 1) * P, s * P:(s + 1) * P])
                    nc.tensor.matmul(ps[:], xt[:], W1sb[k][:],
                                     start=(k == 0), stop=(k == kf - 1))
                nc.scalar.activation(sup_all[:, s * nhid:(s + 1) * nhid],
                                     ps[:], mybir.ActivationFunctionType.Copy)
            # one DMA: [p, s*nhid+f] -> dram[s*128+p, f]
            sup_ap = bass.AP(
                sup_shard[:].tensor, sup_shard[:].offset,
                [[nhid, P], [P * nhid, strips], [1, nhid]])
            nc.sync.dma_start(sup_ap, sup_all[:].rearrange(
                "p (s f) -> p s f", f=nhid))
            nc.gpsimd.collective_compute(
                "AllGather", mybir.AluOpType.bypass, replica_groups=rg,
                ins=[sup_shard[:].opt()], outs=[table1[:].opt()])

            # ---------------- helper: spmm strips ----------------
            def spmm_strip(s, table, is_layer1, h_all, logits_all):
                aux_sb = sbA.tile([P, aux_cols], mybir.dt.int16, tag="aux")
                nc.sync.dma_start(
                    aux_sb[:], aux_d[:, s * aux_cols:(s + 1) * aux_cols])
                gath = sb.tile([P, nch, P], mybir.dt.bfloat16, tag="gath")
                for q in range(4):
                    nc.gpsimd.dma_gather(
                        gath[:, q * cpq:(q + 1) * cpq, :],
                        table[q * qsize:(q + 1) * qsize, :],
                        aux_sb[:, q * idx_cols:(q + 1) * idx_cols],
                        cpq * P, cpq * P, P, queue_num=q)
                # S build
                S_sb = sbS.tile([P, nch * WIN], mybir.dt.bfloat16, tag="S")
                rr_ap = aux_sb[:, 4 * idx_cols:4 * idx_cols + nch].bitcast(
                    mybir.dt.bfloat16)
                val_ap = aux_sb[:, 4 * idx_cols + nch:aux_cols].bitcast(
                    mybir.dt.bfloat16)
                rr_b = bass.AP(rr_ap.tensor, rr_ap.offset,
                               [rr_ap.ap[0], [1, nch], [0, WIN]])
                val_b = bass.AP(val_ap.tensor, val_ap.offset,
                                [val_ap.ap[0], [1, nch], [0, WIN]])
                io_ap = iota_sb[:]
                iota_b = bass.AP(io_ap.tensor, io_ap.offset,
                                 [io_ap.ap[0], [0, nch], [1, WIN]])
                nc.vector.tensor_tensor(out=S_sb[:], in0=rr_b, in1=iota_b,
                                        op=mybir.AluOpType.is_equal)
                nc.vector.tensor_tensor(out=S_sb[:], in0=S_sb[:], in1=val_b,
                                        op=mybir.AluOpType.mult)
                ps = psp.tile([P, P], mybir.dt.float32, space="PSUM",
                              tag="psB")
                nc.tensor.matmul(ps[:], ones_sb[:],
                                 b1row[:] if is_layer1 else zrow_sb[:],
                                 start=True, stop=True, tile_position=(0, 0))
                for ck in range(nch):
                    w = (ck % cpq) // J
                    nc.tensor.matmul(
                        ps[w * WIN:(w + 1) * WIN, :],
                        S_sb[:, ck * WIN:(ck + 1) * WIN],
                        gath[:, ck, :],
                        start=False, stop=False, skip_group_check=True,
                        tile_position=(0, w * WIN))
                if is_layer1:
                    # h strip + phase C: t2 = h @ W2pad
                    nc.scalar.activation(h_all[:, s * P:(s + 1) * P], ps[:],
                                         mybir.ActivationFunctionType.Relu)
                    hT_ps = psC.tile([P, P], mybir.dt.bfloat16, space="PSUM",
                                     tag="psT")
                    nc.tensor.transpose(hT_ps[:],
                                        h_all[:, s * P:(s + 1) * P], ident[:])
                    hT_sb = sb.tile([P, P], mybir.dt.bfloat16, tag="hT")
                    nc.vector.tensor_copy(hT_sb[:], hT_ps[:])
                    t2_ps = psC.tile([P, P], mybir.dt.float32, space="PSUM",
                                     tag="psT2")
                    nc.tensor.matmul(t2_ps[:], hT_sb[:], W2pad[:],
                                     start=True, stop=True)
                    t2_sb = sb.tile([P, P], mybir.dt.bfloat16, tag="t2")
                    nc.scalar.activation(t2_sb[:], t2_ps[:],
                                         mybir.ActivationFunctionType.Copy)
                    t2_ap = bass.AP(
                        t2_shard[:].tensor,
                        t2_shard[:].offset + s * P * P,
                        [[P, P], [1, P]])
                    nc.sync.dma_start(t2_ap, t2_sb[:])
                else:
                    # b2 + log_softmax
                    nc.tensor.matmul(ps[:, 0:ncls], ones_sb[:], b2row[:],
                                     start=False, stop=False,
                                     skip_group_check=True,
                                     tile_position=(0, 0))
                    negmax = sb.tile([P, 1], mybir.dt.float32, tag="nm")
                    nc.vector.reduce_max(out=negmax[:], in_=ps[:, 0:ncls],
                                         axis=mybir.AxisListType.X,
                                         negate=True)
                    esum = sb.tile([P, 1], mybir.dt.float32, tag="es")
                    etile = sb.tile([P, ncls], mybir.dt.float32, tag="et")
                    nc.scalar.activation(etile[:], ps[:, 0:ncls],
                                         mybir.ActivationFunctionType.Exp,
                                         bias=negmax[:], accum_out=esum[:])
                    lse = sb.tile([P, 1], mybir.dt.float32, tag="lse")
                    nc.scalar.activation(lse[:], esum[:],
                                         mybir.ActivationFunctionType.Ln)
                    nc.vector.tensor_scalar(
                        out=logits_all[:, s * ncls:(s + 1) * ncls],
                        in0=ps[:, 0:ncls], scalar1=negmax[:], scalar2=lse[:],
                        op0=mybir.AluOpType.add, op1=mybir.AluOpType.subtract)

            # ---------------- phase B (+C inline) ----------------
            h_all = big.tile([P, strips * P], mybir.dt.bfloat16, tag="big")
            for s in range(strips):
                spmm_strip(s, table1, True, h_all, None)
            nc.gpsimd.collective_compute(
                "AllGather", mybir.AluOpType.bypass, replica_groups=rg,
                ins=[t2_shard[:].opt()], outs=[table2[:].opt()])

            # ---------------- phase D ----------------
            logits_all = big.tile([P, strips * ncls], mybir.dt.float32,
                                  tag="logits")
            for s in range(strips):
                spmm_strip(s, table2, False, None, logits_all)
            out_ap = bass.AP(
                out_d.ap().tensor, 0,
                [[ncls, P], [P * ncls, strips], [1, ncls]])
            nc.sync.dma_start(out_ap, logits_all[:].rearrange(
                "p (s c) -> p s c", c=ncls))

    nc.compile()
    return nc


def _get_program(cfg):
    key = tuple(sorted(cfg.items()))
    if key not in _PROGRAM_CACHE:
        _PROGRAM_CACHE[key] = _build_program(cfg)
    return _PROGRAM_CACHE[key]


# ----------------------------------------------------------------------------
# entry point
# ----------------------------------------------------------------------------

def kernel(x, adj_row, adj_col, adj_val, i=None, W1=None, b1=None, W2=None,
           b2=None, _cfg=None, _trace=False, **_ignored):
    from concourse.bass_utils import run_bass_kernel_spmd

    x = np.asarray(x, np.float32)
    adj_row = np.asarray(adj_row, np.int32)
    adj_col = np.asarray(adj_col, np.int32)
    adj_val = np.asarray(adj_val, np.float32)
    W1 = np.asarray(W1, np.float32)
    b1 = np.asarray(b1, np.float32).reshape(1, -1)
    W2 = np.asarray(W2, np.float32)
    b2 = np.asarray(b2, np.float32).reshape(1, -1)

    cfg = dict(_cfg or DEFAULT_CFG)
    cfg.setdefault("nfeat", x.shape[1])
    cfg.setdefault("nhid", W1.shape[1])
    cfg.setdefault("ncls", W2.shape[1])

    aux, xT, pi, cfg = _prep(x, adj_row, adj_col, adj_val, cfg)
    nc = _get_program(cfg)

    n_cores = cfg["n_cores"]
    iota = np.tile(np.arange(WIN, dtype=np.float32)[None, :],
                   (P, 1)).astype(BF16)
    in_maps = [
        dict(aux=aux[c], xT=xT[c], W1=W1, W2=W2, b1=b1, b2=b2, iota=iota)
        for c in range(n_cores)
    ]
    res = run_bass_kernel_spmd(nc, in_maps, core_ids=list(range(n_cores)),
                               trace=_trace)
    out_all = np.concatenate([r["out"] for r in res.results], 0)
    result = out_all[pi]
    if _trace:
        kernel.last_results = res
    return result.astype(np.float32)
